# revision 1
# baseline (speedup 1.0000x reference)
"""COO SpMM (out[r] = sum_e A_val[e] * x[col_e] for row_e == r) on 8 Trainium2
NeuronCores.

Strategy (row-block sharding, single SPMD NEFF):
- Each core owns a contiguous block of output rows (N/8). Host buckets edges
  by (core, 128-row window, col chunk), pads each (window, chunk) group to a
  multiple of 128 edges.  Group sizes are the max over cores so that one
  static program serves all 8 cores; per-core shortfall is expressed as
  trailing -1 gather indices (skipped by the DMA ucode) plus a per-call
  valid-count register.
- Device per 128-edge batch: dma_gather pulls the 128 x-rows (512B each) into
  a [128 edge, 128 feat] SBUF tile; the vector engine builds the scaled
  one-hot S[e, r] = (iota==rloc[e])*val[e] in one fused tensor_scalar; the
  tensor engine accumulates S^T @ C into the window's PSUM tile.  Per window,
  the scalar engine copies PSUM->SBUF and a HWDGE DMA writes the output rows.
- Gather indices are int16 (hardware requirement), so x's row space is split
  into 4 chunks of 25000 rows; a gather call's base pointer selects the chunk.
"""
import math

import numpy as np

P = 128           # partitions / matmul K / window rows
MAXI = 1024       # max idxs per dma_gather call (SWDGE ring limit)
MAXB = MAXI // P  # max batches per gather call


class Plan:
    """Static program structure shared by all cores (derived from counts)."""

    def __init__(self, n, nnz, f, n_cores, n_chunks, counts):
        # counts: [n_cores, n_windows, n_chunks] edge counts
        self.n, self.nnz, self.f = n, nnz, f
        self.n_cores, self.n_chunks = n_cores, n_chunks
        self.rpc = n // n_cores                      # rows per core
        self.n_windows = math.ceil(self.rpc / P)
        self.rpc_pad = self.n_windows * P
        self.chunk_rows = math.ceil(n / n_chunks)
        assert self.chunk_rows < 2 ** 15
        # batches per (window, chunk) group: max over cores, >= 1
        self.bg = np.maximum(
            1, np.ceil(counts.max(axis=0) / P).astype(np.int64)
        )  # [n_windows, n_chunks]
        # slot offsets (in batches) per group
        self.gslot = np.zeros((self.n_windows, self.n_chunks), dtype=np.int64)
        acc = 0
        self.calls = []  # (w, c, slot_b, nb) slot_b = batch offset in stream
        for w in range(self.n_windows):
            for c in range(self.n_chunks):
                self.gslot[w, c] = acc
                b = int(self.bg[w, c])
                off = 0
                while off < b:
                    nb = min(MAXB, b - off)
                    self.calls.append((w, c, acc + off, nb))
                    off += nb
                acc += b
        self.total_batches = acc
        self.slots = acc * P


def _plan_and_pack(x, row, col, val, n_cores, n_chunks):
    """Host-side: bucket/sort edges, build per-core packed streams."""
    n, f = x.shape
    nnz = len(val)
    rpc = n // n_cores
    core = row // rpc
    rr = row % rpc
    w = rr // P
    chunk_rows = math.ceil(n / n_chunks)
    c = col // chunk_rows

    n_windows = math.ceil(rpc / P)
    counts = np.zeros((n_cores, n_windows, n_chunks), dtype=np.int64)
    np.add.at(counts, (core, w, c), 1)

    plan = Plan(n, nnz, f, n_cores, n_chunks, counts)

    # sort edges by (core, w, c); within group order arbitrary
    order = np.lexsort((c, w, core))
    s_core, s_w, s_c = core[order], w[order], c[order]
    s_col, s_val, s_rloc = col[order], val[order], (rr % P)[order]

    # slot position for each edge: group base + index within group
    gb = plan.gslot[s_w, s_c] * P  # slot base per edge's group
    # index within group: cumulative position among same (core,w,c)
    # counts are per (core,w,c); use the sorted order to compute offsets
    key = (s_core * n_windows + s_w) * n_chunks + s_c
    # start index of each key run in the sorted array
    run_starts = np.searchsorted(key, np.arange(key.max() + 1) + 0, side="left")
    within = np.arange(nnz) - run_starts[key]

    slot = gb + within  # per-core slot index (0..plan.slots)

    idx16_local = (s_col - s_c * chunk_rows).astype(np.int16)

    per_core = []
    ncalls = len(plan.calls)
    # precompute per-call slot ranges
    call_b = np.array([cb for (_, _, cb, _) in plan.calls], dtype=np.int64)
    call_nb = np.array([nb for (_, _, _, nb) in plan.calls], dtype=np.int64)

    for ci in range(n_cores):
        m = s_core == ci
        sl = slot[m]
        vals = np.zeros(plan.slots, dtype=np.float32)
        rlocs = np.zeros(plan.slots, dtype=np.float32)
        idxs = np.full(plan.slots, -1, dtype=np.int16)
        vals[sl] = s_val[m]
        rlocs[sl] = s_rloc[m]
        idxs[sl] = idx16_local[m]
        # per-group: valid edges are packed at the group's start; pad tail -1.
        # Ensure every call has >= 1 valid idx (sim requirement, harmless on hw)
        gcnt = np.zeros(ncalls, dtype=np.int32)
        for k in range(ncalls):
            a = call_b[k] * P
            b = a + call_nb[k] * P
            blk = idxs[a:b]
            nv = int((blk >= 0).sum())
            if nv == 0:
                blk[0] = 0
                nv = 1
            # valid entries must be a prefix (they are: packed from group base,
            # and calls split the group contiguously)
            assert (blk[:nv] >= 0).all() and (blk[nv:] == -1).all()
            gcnt[k] = nv
        # wrap idxs per call: position k -> [k%16, k//16], replicate to 128 p
        blocks = []
        for k in range(ncalls):
            a = call_b[k] * P
            b = a + call_nb[k] * P
            v = idxs[a:b]
            blocks.append(np.tile(v.reshape(-1, 16).T, (8, 1)))
        idxw = np.concatenate(blocks, axis=1).astype(np.int16)  # [128, slots/16]
        per_core.append({
            "idxw": idxw,
            "val": vals.reshape(-1, P).T.copy(),    # [128, total_batches]
            "rloc": rlocs.reshape(-1, P).T.copy(),  # [128, total_batches]
            "gcnt": gcnt.reshape(1, -1),
        })
    return plan, per_core


def _build_program(plan):
    import concourse.bacc as bacc
    import concourse.mybir as mybir
    from concourse.tile import TileContext
    from concourse.library_config import mlp

    f = plan.f
    nb_tot = plan.total_batches
    ncalls = len(plan.calls)

    nc = bacc.Bacc(None, target_bir_lowering=False, debug=False)
    x_d = nc.dram_tensor("x", [plan.n, f], mybir.dt.float32, kind="ExternalInput")
    iota_d = nc.dram_tensor("iota", [P, P], mybir.dt.float32, kind="ExternalInput")
    idx_d = nc.dram_tensor("idxw", [P, plan.slots // 16], mybir.dt.int16,
                           kind="ExternalInput")
    val_d = nc.dram_tensor("val", [P, nb_tot], mybir.dt.float32, kind="ExternalInput")
    rloc_d = nc.dram_tensor("rloc", [P, nb_tot], mybir.dt.float32, kind="ExternalInput")
    gcnt_d = nc.dram_tensor("gcnt", [1, ncalls], mybir.dt.int32, kind="ExternalInput")
    out_d = nc.dram_tensor("out", [plan.rpc_pad, f], mybir.dt.float32,
                           kind="ExternalOutput")

    # calls grouped per window for scheduling
    calls_by_w = [[] for _ in range(plan.n_windows)]
    for k, (w, c, cb, nb) in enumerate(plan.calls):
        calls_by_w[w].append((k, c, cb, nb))

    with TileContext(nc) as tc:
        with tc.tile_pool(name="sbuf", bufs=1) as spool, \
             tc.tile_pool(name="sel", bufs=4) as selpool, \
             tc.tile_pool(name="stage", bufs=3) as stpool, \
             tc.tile_pool(name="psum", bufs=2, space="PSUM") as ppool:
            iota_t = spool.tile([P, P], mybir.dt.float32)
            idx_t = spool.tile([P, plan.slots // 16], mybir.dt.int16)
            val_t = spool.tile([P, nb_tot], mybir.dt.float32)
            rloc_t = spool.tile([P, nb_tot], mybir.dt.float32)
            gcnt_t = spool.tile([1, ncalls], mybir.dt.int32)
            # persistent rotating gather buffers; memset once so that slots
            # never written by a gather (trailing pads) stay finite (0 x NaN
            # would poison the matmul otherwise)
            NCB = 8
            cts = [spool.tile([P, MAXB, f], mybir.dt.float32, name=f"cb{i}")
                   for i in range(NCB)]
            for i in range(NCB):
                nc.vector.memset(cts[i][:], 0.0)
            nc.sync.dma_start(out=iota_t[:], in_=iota_d[:])
            nc.sync.dma_start(out=idx_t[:], in_=idx_d[:])
            nc.sync.dma_start(out=val_t[:], in_=val_d[:])
            nc.sync.dma_start(out=rloc_t[:], in_=rloc_d[:])
            nc.sync.dma_start(out=gcnt_t[:], in_=gcnt_d[:])
            nc.gpsimd.load_library(mlp)
            nreg = nc.gpsimd.alloc_register("nidx")

            ci_rot = 0
            for w in range(plan.n_windows):
                wcalls = calls_by_w[w]
                ctiles = []
                for (k, c, cb, nb) in wcalls:
                    c_t = cts[ci_rot % NCB]
                    ci_rot += 1
                    nc.gpsimd.reg_load(nreg, gcnt_t[0:1, k:k + 1])
                    nc.gpsimd.dma_gather(
                        c_t[:, :nb, :],
                        x_d[c * plan.chunk_rows:
                            min((c + 1) * plan.chunk_rows, plan.n)],
                        idx_t[:, cb * 8:(cb + nb) * 8],
                        nb * P, nreg, f,
                    )
                    ctiles.append((c_t, cb, nb))
                psum_t = ppool.tile([P, f], mybir.dt.float32, name=f"ps{w}",
                                    tag=f"ps{w % 2}", space="PSUM")
                nbat = sum(nb for (_, _, nb) in ctiles)
                bi = 0
                for (c_t, cb, nb) in ctiles:
                    for b in range(nb):
                        sb = cb + b  # global batch slot
                        s_t = selpool.tile([P, P], mybir.dt.float32, name=f"s{sb}",
                                           tag=f"s{sb % 4}")
                        nc.vector.tensor_scalar(
                            out=s_t[:], in0=iota_t[:],
                            scalar1=rloc_t[:, sb:sb + 1],
                            scalar2=val_t[:, sb:sb + 1],
                            op0=mybir.AluOpType.is_equal,
                            op1=mybir.AluOpType.mult,
                        )
                        nc.tensor.matmul(
                            out=psum_t[:], lhsT=s_t[:], rhs=c_t[:, b, :],
                            start=(bi == 0), stop=(bi == nbat - 1),
                        )
                        bi += 1
                st_t = stpool.tile([P, f], mybir.dt.float32, name=f"st{w}",
                                   tag=f"st{w % 3}")
                nc.scalar.copy(out=st_t[:], in_=psum_t[:])
                nc.sync.dma_start(out=out_d[w * P:(w + 1) * P], in_=st_t[:])
    nc.compile()
    return nc


def _run(nc, plan, x, per_core, n_cores):
    from concourse.bass_utils import run_bass_kernel_spmd
    iota = np.tile(np.arange(P, dtype=np.float32)[None, :], (P, 1))
    in_maps = []
    for ci in range(n_cores):
        pc = per_core[ci]
        in_maps.append({
            "x": x, "iota": iota, "idxw": pc["idxw"], "val": pc["val"],
            "rloc": pc["rloc"], "gcnt": pc["gcnt"],
        })
    res = run_bass_kernel_spmd(nc, in_maps, core_ids=list(range(n_cores)))
    rpc = plan.rpc
    return np.concatenate([res.results[ci]["out"][:rpc] for ci in range(n_cores)],
                          axis=0)


_PROGRAM_CACHE = {}


def spmm(x, A_ind, A_val, n_cores=8, n_chunks=4):
    x = np.asarray(x, dtype=np.float32)
    row = np.asarray(A_ind[0], dtype=np.int64)
    col = np.asarray(A_ind[1], dtype=np.int64)
    val = np.asarray(A_val, dtype=np.float32)
    plan, per_core = _plan_and_pack(x, row, col, val, n_cores, n_chunks)
    # the compiled program depends only on the plan structure; reuse it when
    # kernel() is called repeatedly with same-shaped (or identical) inputs
    key = (x.shape, plan.n_chunks, plan.n_cores, plan.bg.tobytes())
    nc = _PROGRAM_CACHE.get(key)
    if nc is None:
        nc = _build_program(plan)
        _PROGRAM_CACHE.clear()
        _PROGRAM_CACHE[key] = nc
    return _run(nc, plan, x, per_core, n_cores)


def kernel(x, A_ind, A_val):
    return spmm(np.asarray(x), np.asarray(A_ind), np.asarray(A_val))



# revision 10
# speedup vs baseline: 1.0017x; 1.0017x over previous
"""COO SpMM (out[r] = sum_e A_val[e] * x[col_e] for row_e == r) on 8 Trainium2
NeuronCores.

Strategy (row-block sharding, single SPMD NEFF):
- Each core owns a contiguous block of output rows (N/8 = 12500).  Host
  buckets edges by (core, 128-row window, col chunk) and pads each (window,
  chunk) group to a multiple of 128 edges (group sizes are the max over
  cores so one static program serves all 8 cores; pad slots use idx=0 with
  val=0, so they gather harmless data that the one-hot matmul zeroes out).
- x is cast to bf16 on the host (rel-err budget 2e-2 >> bf16's ~2e-3):
  halves the gather traffic, enables fast-weight-load bf16 matmuls, and
  doubles DVE throughput for the one-hot build.
- Groups are laid out chunk-major: each chunk owns one contiguous batch
  stream ordered by window.  Gathers are issued as large 4096-index calls
  spanning many (window, chunk) groups, amortizing the ~1us/call SWDGE
  descriptor-generation overhead (the old per-group calls paid it 778x).
- Device per 128-edge batch: the vector engine builds the scaled one-hot
  S[e, r] = (iota==rloc[e])*val[e] (bf16) in one fused tensor_scalar; the
  tensor engine accumulates S^T @ C into the window's PSUM tile (fp32).
  Per window, the scalar engine copies PSUM->SBUF and a HWDGE DMA writes
  the output rows (fp32).
- Gather indices are int16 (hardware requirement), so x's row space is
  split into 4 chunks of 25000 rows; a gather call's base pointer selects
  the chunk.
"""
import math

import numpy as np
import ml_dtypes

P = 128            # partitions / matmul K / window rows
CALLB = 32         # batches per dma_gather call (4096 indices)
N_CORES = 8
N_CHUNKS = 4
BF16 = ml_dtypes.bfloat16


class Plan:
    """Static program structure shared by all cores (derived from counts)."""

    def __init__(self, n, nnz, f, counts):
        # counts: [n_cores, n_windows, n_chunks] edge counts
        self.n, self.nnz, self.f = n, nnz, f
        self.rpc = n // N_CORES                      # rows per core
        self.n_windows = math.ceil(self.rpc / P)
        self.rpc_pad = self.n_windows * P
        self.chunk_rows = math.ceil(n / N_CHUNKS)
        assert self.chunk_rows < 2 ** 15
        # batches per (window, chunk) group: max over cores, >= 1
        self.bg = np.maximum(
            1, np.ceil(counts.max(axis=0) / P).astype(np.int64)
        )  # [n_windows, n_chunks]
        # chunk-major batch layout: chunk c owns batches
        # [chunk_base[c], chunk_base[c] + chunk_nb[c]), ordered by window.
        self.chunk_nb = self.bg.sum(axis=0)          # [n_chunks]
        self.chunk_base = np.concatenate([[0], np.cumsum(self.chunk_nb)])
        self.total_batches = int(self.chunk_nb.sum())
        self.slots = self.total_batches * P
        # group (w, c) -> global batch offset
        self.gslot = np.zeros((self.n_windows, N_CHUNKS), dtype=np.int64)
        for c in range(N_CHUNKS):
            self.gslot[:, c] = self.chunk_base[c] + np.concatenate(
                [[0], np.cumsum(self.bg[:-1, c])])
        # gather calls: spans of <= CALLB batches within each chunk stream
        self.calls = []  # (chunk, global_batch0, nb, first_window, last_window)
        for c in range(N_CHUNKS):
            # window boundaries within this chunk's stream (batch offsets)
            wb = np.concatenate([[0], np.cumsum(self.bg[:, c])])
            off = 0
            while off < self.chunk_nb[c]:
                nb = min(CALLB, int(self.chunk_nb[c]) - off)
                fw = int(np.searchsorted(wb, off, side="right") - 1)
                lw = int(np.searchsorted(wb, off + nb - 1, side="right") - 1)
                self.calls.append(
                    (c, int(self.chunk_base[c]) + off, nb, fw, lw))
                off += nb
        # call lookup: for chunk c, call j covers batches [j*CALLB, ...)
        self.calls_by_chunk = [[] for _ in range(N_CHUNKS)]
        for k, (c, b0, nb, fw, lw) in enumerate(self.calls):
            self.calls_by_chunk[c].append((k, b0, nb, fw, lw))


def _plan_and_pack(x, row, col, val):
    """Host-side: bucket/sort edges, build per-core packed streams."""
    n, f = x.shape
    nnz = len(val)
    rpc = n // N_CORES
    core = row // rpc
    rr = row % rpc
    w = rr // P
    chunk_rows = math.ceil(n / N_CHUNKS)
    c = col // chunk_rows

    n_windows = math.ceil(rpc / P)
    counts = np.zeros((N_CORES, n_windows, N_CHUNKS), dtype=np.int64)
    np.add.at(counts, (core, w, c), 1)

    plan = Plan(n, nnz, f, counts)

    # sort edges by (core, c, w): chunk-major streams ordered by window
    order = np.lexsort((w, c, core))
    s_core, s_w, s_c = core[order], w[order], c[order]
    s_col, s_val, s_rloc = col[order], val[order], (rr % P)[order]

    # slot position for each edge: group base + index within group
    gb = plan.gslot[s_w, s_c] * P
    key = (s_core * N_CHUNKS + s_c) * n_windows + s_w
    run_starts = np.searchsorted(key, np.arange(key.max() + 1), side="left")
    within = np.arange(nnz) - run_starts[key]
    slot = gb + within  # per-core slot index (0..plan.slots)

    idx16_local = (s_col - s_c * chunk_rows).astype(np.int16)

    per_core = []
    for ci in range(N_CORES):
        m = s_core == ci
        sl = slot[m]
        vals = np.zeros(plan.slots, dtype=np.float32)
        rlocs = np.zeros(plan.slots, dtype=np.float32)
        idxs = np.zeros(plan.slots, dtype=np.int16)  # pad -> row 0, val 0
        vals[sl] = s_val[m]
        rlocs[sl] = s_rloc[m]
        idxs[sl] = idx16_local[m]
        # wrap idxs per call: position k -> [k%16, k//16], replicate to 128 p
        blocks = []
        for (c_, b0, nb, fw, lw) in plan.calls:
            a = b0 * P
            b = a + nb * P
            v = idxs[a:b]
            blocks.append(np.tile(v.reshape(-1, 16).T, (8, 1)))
        idxw = np.concatenate(blocks, axis=1).astype(np.int16)  # [128, slots/16]
        per_core.append({
            "idxw": idxw,
            "val": vals.reshape(-1, P).T.copy(),    # [128, total_batches]
            "rloc": rlocs.reshape(-1, P).T.copy(),  # [128, total_batches]
        })
    return plan, per_core


def _build_program(plan, mode="full"):
    # mode: "full" | "gather_only" | "compute_only"  (ablation timing aids)
    import concourse.bacc as bacc
    import concourse.mybir as mybir
    from concourse.tile import TileContext
    from concourse.library_config import mlp

    f = plan.f
    nb_tot = plan.total_batches

    nc = bacc.Bacc(None, target_bir_lowering=False, debug=False)
    x_d = nc.dram_tensor("x", [plan.n, f], mybir.dt.bfloat16,
                         kind="ExternalInput")
    iota_d = nc.dram_tensor("iota", [P, P], mybir.dt.bfloat16,
                            kind="ExternalInput")
    idx_d = nc.dram_tensor("idxw", [P, plan.slots // 16], mybir.dt.int16,
                           kind="ExternalInput")
    val_d = nc.dram_tensor("val", [P, nb_tot], mybir.dt.float32,
                           kind="ExternalInput")
    rloc_d = nc.dram_tensor("rloc", [P, nb_tot], mybir.dt.float32,
                            kind="ExternalInput")
    out_d = nc.dram_tensor("out", [plan.rpc_pad, f], mybir.dt.float32,
                           kind="ExternalOutput")

    NBUF = 3     # gather tiles per chunk

    with TileContext(nc) as tc:
        with tc.tile_pool(name="sbuf", bufs=1) as spool, \
             tc.tile_pool(name="sel", bufs=4) as selpool, \
             tc.tile_pool(name="stage", bufs=3) as stpool, \
             tc.tile_pool(name="psum", bufs=2, space="PSUM") as ppool:
            iota_t = spool.tile([P, P], mybir.dt.bfloat16)
            idx_t = spool.tile([P, plan.slots // 16], mybir.dt.int16)
            val_t = spool.tile([P, nb_tot], mybir.dt.float32)
            rloc_t = spool.tile([P, nb_tot], mybir.dt.float32)
            cts = [[spool.tile([P, CALLB, f], mybir.dt.bfloat16,
                               name=f"cb{c}_{i}") for i in range(NBUF)]
                   for c in range(N_CHUNKS)]
            nc.sync.dma_start(out=iota_t[:], in_=iota_d[:])
            nc.sync.dma_start(out=idx_t[:], in_=idx_d[:])
            nc.sync.dma_start(out=val_t[:], in_=val_d[:])
            nc.sync.dma_start(out=rloc_t[:], in_=rloc_d[:])
            nc.gpsimd.load_library(mlp)

            # per-chunk cursor into calls_by_chunk (next unissued call)
            next_call = [0] * N_CHUNKS
            # idxw column offset per call (CALLB batches -> CALLB*8 cols)
            call_col = {}
            acc = 0
            for k, (c_, b0, nb, fw, lw) in enumerate(plan.calls):
                call_col[k] = acc
                acc += nb * 8

            def issue(c, j):
                if mode == "compute_only":
                    return
                k, b0, nb, fw, lw = plan.calls_by_chunk[c][j]
                c_t = cts[c][j % NBUF]
                col0 = call_col[k]
                nc.gpsimd.dma_gather(
                    c_t[:, :nb, :],
                    x_d[c * plan.chunk_rows:
                        min((c + 1) * plan.chunk_rows, plan.n)],
                    idx_t[:, col0:col0 + nb * 8],
                    nb * P, nb * P, f,
                    single_packet=False,
                )

            def emit_eligible(c, w_done):
                # Emit the next gather call(s) for chunk c.  A call may only
                # be emitted once every consumer of the tile it overwrites
                # (call j-NBUF) has been emitted, i.e. after that call's last
                # reader window has been fully generated (w_done).
                cbc = plan.calls_by_chunk[c]
                while next_call[c] < len(cbc):
                    j = next_call[c]
                    if j >= NBUF and cbc[j - NBUF][4] > w_done:
                        break
                    issue(c, j)
                    next_call[c] += 1

            if mode == "compute_only":
                for c in range(N_CHUNKS):
                    for i in range(NBUF):
                        nc.vector.memset(cts[c][i][:], 0.0)
            zst_t = None
            if mode == "gather_only":
                zst_t = stpool.tile([P, f], mybir.dt.float32, name="zst")
                nc.vector.memset(zst_t[:], 0.0)

            for c in range(N_CHUNKS):
                emit_eligible(c, -1)   # prefetch first NBUF calls per chunk

            for w in range(plan.n_windows):
                for c in range(N_CHUNKS):
                    # the data for window w must already be emitted
                    cbc = plan.calls_by_chunk[c]
                    need = int(plan.gslot[w, c] + plan.bg[w, c] - 1
                               - plan.chunk_base[c]) // CALLB
                    assert need < next_call[c], (
                        f"w={w} c={c}: call {need} not emitted "
                        f"(next={next_call[c]}); increase NBUF or CALLB")
                if mode == "gather_only":
                    nc.sync.dma_start(out=out_d[w * P:(w + 1) * P],
                                      in_=zst_t[:])
                    for c in range(N_CHUNKS):
                        emit_eligible(c, w)
                    continue
                psum_t = ppool.tile([P, f], mybir.dt.float32, name=f"ps{w}",
                                    tag=f"ps{w % 4}", space="PSUM")
                nbat = int(plan.bg[w].sum())
                bi = 0
                for c in range(N_CHUNKS):
                    g0 = int(plan.gslot[w, c])
                    for B in range(g0, g0 + int(plan.bg[w, c])):
                        rel = B - int(plan.chunk_base[c])
                        j, loc = rel // CALLB, rel % CALLB
                        c_t = cts[c][j % NBUF]
                        s_t = selpool.tile([P, P], mybir.dt.bfloat16,
                                           name=f"s{B}", tag=f"s{B % 4}")
                        nc.vector.tensor_scalar(
                            out=s_t[:], in0=iota_t[:],
                            scalar1=rloc_t[:, B:B + 1],
                            scalar2=val_t[:, B:B + 1],
                            op0=mybir.AluOpType.is_equal,
                            op1=mybir.AluOpType.mult,
                        )
                        nc.tensor.matmul(
                            out=psum_t[:], lhsT=s_t[:], rhs=c_t[:, loc, :],
                            start=(bi == 0), stop=(bi == nbat - 1),
                        )
                        bi += 1
                st_t = stpool.tile([P, f], mybir.dt.float32, name=f"st{w}",
                                   tag=f"st{w % 3}")
                nc.scalar.copy(out=st_t[:], in_=psum_t[:])
                nc.sync.dma_start(out=out_d[w * P:(w + 1) * P], in_=st_t[:])
                for c in range(N_CHUNKS):
                    emit_eligible(c, w)
    nc.compile()
    return nc


def _make_inputs(plan, x, per_core):
    iota = np.tile(np.arange(P, dtype=np.float32)[None, :], (P, 1)).astype(BF16)
    xb = np.ascontiguousarray(x.astype(BF16))
    in_maps = []
    for ci in range(N_CORES):
        pc = per_core[ci]
        in_maps.append({
            "x": xb, "iota": iota, "idxw": pc["idxw"], "val": pc["val"],
            "rloc": pc["rloc"],
        })
    return in_maps


def _run(nc, plan, x, per_core):
    from concourse.bass_utils import run_bass_kernel_spmd
    in_maps = _make_inputs(plan, x, per_core)
    res = run_bass_kernel_spmd(nc, in_maps, core_ids=list(range(N_CORES)))
    rpc = plan.rpc
    return np.concatenate(
        [res.results[ci]["out"][:rpc] for ci in range(N_CORES)], axis=0)


_PROGRAM_CACHE = {}


def spmm(x, A_ind, A_val):
    x = np.asarray(x, dtype=np.float32)
    row = np.asarray(A_ind[0], dtype=np.int64)
    col = np.asarray(A_ind[1], dtype=np.int64)
    val = np.asarray(A_val, dtype=np.float32)
    plan, per_core = _plan_and_pack(x, row, col, val)
    key = (x.shape, plan.bg.tobytes())
    nc = _PROGRAM_CACHE.get(key)
    if nc is None:
        nc = _build_program(plan)
        _PROGRAM_CACHE.clear()
        _PROGRAM_CACHE[key] = nc
    return _run(nc, plan, x, per_core)


def kernel(x, A_ind, A_val):
    return spmm(np.asarray(x), np.asarray(A_ind), np.asarray(A_val))


# revision 19
# speedup vs baseline: 1.6028x; 1.6001x over previous
"""COO SpMM (out[r] = sum_e A_val[e] * x[col_e] for row_e == r) on 8 Trainium2
NeuronCores.

Strategy (row-block sharding, single SPMD NEFF):
- Each core owns a contiguous block of output rows (N/8 = 12500).  Host
  buckets edges by (core, 128-row window, col chunk) and pads each (window,
  chunk) group to a multiple of 128 edges (group sizes are the max over
  cores so one static program serves all 8 cores; pad slots use idx=0 with
  val=0, so they gather harmless data that the one-hot matmul zeroes out).
- x is cast to bf16 on the host (rel-err budget 2e-2 >> bf16's ~2e-3):
  halves the gather traffic, enables fast-weight-load bf16 matmuls, and
  doubles DVE throughput for the one-hot build.
- Groups are laid out chunk-major: each chunk owns one contiguous batch
  stream ordered by window.  Gathers are issued as large 4096-index calls
  spanning many (window, chunk) groups, amortizing the ~1us/call SWDGE
  descriptor-generation overhead (the old per-group calls paid it 778x).
- Device per 128-edge batch: the vector engine builds the scaled one-hot
  S[e, r] = (iota==rloc[e])*val[e] (bf16) in one fused tensor_scalar; the
  tensor engine accumulates S^T @ C into the window's PSUM tile (fp32).
  Per window, the scalar engine copies PSUM->SBUF and a HWDGE DMA writes
  the output rows (fp32).
- Gather indices are int16 (hardware requirement), so x's row space is
  split into 4 chunks of 25000 rows; a gather call's base pointer selects
  the chunk.
"""
import math

import numpy as np
import ml_dtypes

P = 128            # partitions / matmul K / window rows
CALLB = 32         # batches per dma_gather call (4096 indices)
N_CORES = 8
N_CHUNKS = 4
BF16 = ml_dtypes.bfloat16


class Plan:
    """Static program structure shared by all cores (derived from counts)."""

    def __init__(self, n, nnz, f, counts):
        # counts: [n_cores, n_windows, n_chunks] edge counts
        self.n, self.nnz, self.f = n, nnz, f
        self.rpc = n // N_CORES                      # rows per core
        self.n_windows = math.ceil(self.rpc / P)
        self.rpc_pad = self.n_windows * P
        self.chunk_rows = math.ceil(n / N_CHUNKS)
        assert self.chunk_rows < 2 ** 15
        # batches per (window, chunk) group: max over cores, >= 1
        self.bg = np.maximum(
            1, np.ceil(counts.max(axis=0) / P).astype(np.int64)
        )  # [n_windows, n_chunks]
        # chunk-major batch layout: chunk c owns batches
        # [chunk_base[c], chunk_base[c] + chunk_nb[c]), ordered by window.
        self.chunk_nb = self.bg.sum(axis=0)          # [n_chunks]
        self.chunk_base = np.concatenate([[0], np.cumsum(self.chunk_nb)])
        self.total_batches = int(self.chunk_nb.sum())
        self.slots = self.total_batches * P
        # group (w, c) -> global batch offset
        self.gslot = np.zeros((self.n_windows, N_CHUNKS), dtype=np.int64)
        for c in range(N_CHUNKS):
            self.gslot[:, c] = self.chunk_base[c] + np.concatenate(
                [[0], np.cumsum(self.bg[:-1, c])])
        # gather calls: spans of <= CALLB batches within each chunk stream
        self.calls = []  # (chunk, global_batch0, nb, first_window, last_window)
        for c in range(N_CHUNKS):
            # window boundaries within this chunk's stream (batch offsets)
            wb = np.concatenate([[0], np.cumsum(self.bg[:, c])])
            off = 0
            while off < self.chunk_nb[c]:
                nb = min(CALLB, int(self.chunk_nb[c]) - off)
                fw = int(np.searchsorted(wb, off, side="right") - 1)
                lw = int(np.searchsorted(wb, off + nb - 1, side="right") - 1)
                self.calls.append(
                    (c, int(self.chunk_base[c]) + off, nb, fw, lw))
                off += nb
        # call lookup: for chunk c, call j covers batches [j*CALLB, ...)
        self.calls_by_chunk = [[] for _ in range(N_CHUNKS)]
        for k, (c, b0, nb, fw, lw) in enumerate(self.calls):
            self.calls_by_chunk[c].append((k, b0, nb, fw, lw))


def _plan_and_pack(x, row, col, val):
    """Host-side: bucket/sort edges, build per-core packed streams."""
    n, f = x.shape
    nnz = len(val)
    rpc = n // N_CORES
    core = row // rpc
    rr = row % rpc
    w = rr // P
    chunk_rows = math.ceil(n / N_CHUNKS)
    c = col // chunk_rows

    n_windows = math.ceil(rpc / P)
    counts = np.zeros((N_CORES, n_windows, N_CHUNKS), dtype=np.int64)
    np.add.at(counts, (core, w, c), 1)

    plan = Plan(n, nnz, f, counts)

    # sort edges by (core, c, w): chunk-major streams ordered by window
    order = np.lexsort((w, c, core))
    s_core, s_w, s_c = core[order], w[order], c[order]
    s_col, s_val, s_rloc = col[order], val[order], (rr % P)[order]

    # slot position for each edge: group base + index within group
    gb = plan.gslot[s_w, s_c] * P
    key = (s_core * N_CHUNKS + s_c) * n_windows + s_w
    run_starts = np.searchsorted(key, np.arange(key.max() + 1), side="left")
    within = np.arange(nnz) - run_starts[key]
    slot = gb + within  # per-core slot index (0..plan.slots)

    idx16_local = (s_col - s_c * chunk_rows).astype(np.int16)

    per_core = []
    for ci in range(N_CORES):
        m = s_core == ci
        sl = slot[m]
        vals = np.zeros(plan.slots, dtype=np.float32)
        rlocs = np.zeros(plan.slots, dtype=np.float32)
        idxs = np.zeros(plan.slots, dtype=np.int16)  # pad -> row 0, val 0
        vals[sl] = s_val[m]
        rlocs[sl] = s_rloc[m]
        idxs[sl] = idx16_local[m]
        # wrap idxs per call: position k -> [k%16, k//16], replicate to 128 p
        blocks = []
        for (c_, b0, nb, fw, lw) in plan.calls:
            a = b0 * P
            b = a + nb * P
            v = idxs[a:b]
            blocks.append(np.tile(v.reshape(-1, 16).T, (8, 1)))
        idxw = np.concatenate(blocks, axis=1).astype(np.int16)  # [128, slots/16]
        per_core.append({
            "idxw": idxw,
            "val": vals.reshape(-1, P).T.copy(),    # [128, total_batches]
            "rloc": rlocs.reshape(-1, P).T.copy(),  # [128, total_batches]
        })
    return plan, per_core


def _build_program(plan, mode="full"):
    # mode: "full" | "gather_only" | "compute_only"  (ablation timing aids)
    import concourse.bacc as bacc
    import concourse.mybir as mybir
    from concourse.tile import TileContext
    from concourse.library_config import mlp

    f = plan.f
    nb_tot = plan.total_batches

    nc = bacc.Bacc(None, target_bir_lowering=False, debug=False,
                   num_swdge_queues=4)
    x_d = nc.dram_tensor("x", [plan.n, f], mybir.dt.bfloat16,
                         kind="ExternalInput")
    iota_d = nc.dram_tensor("iota", [P, P], mybir.dt.bfloat16,
                            kind="ExternalInput")
    idx_d = nc.dram_tensor("idxw", [P, plan.slots // 16], mybir.dt.int16,
                           kind="ExternalInput")
    val_d = nc.dram_tensor("val", [P, nb_tot], mybir.dt.float32,
                           kind="ExternalInput")
    rloc_d = nc.dram_tensor("rloc", [P, nb_tot], mybir.dt.float32,
                            kind="ExternalInput")
    out_d = nc.dram_tensor("out", [plan.rpc_pad, f], mybir.dt.float32,
                           kind="ExternalOutput")

    NBUF = 3     # gather tiles per chunk

    with TileContext(nc) as tc:
        with tc.tile_pool(name="sbuf", bufs=1) as spool, \
             tc.tile_pool(name="sel", bufs=4) as selpool, \
             tc.tile_pool(name="stage", bufs=3) as stpool, \
             tc.tile_pool(name="psum", bufs=2, space="PSUM") as ppool:
            iota_t = spool.tile([P, P], mybir.dt.bfloat16)
            idx_t = spool.tile([P, plan.slots // 16], mybir.dt.int16)
            val_t = spool.tile([P, nb_tot], mybir.dt.float32)
            rloc_t = spool.tile([P, nb_tot], mybir.dt.float32)
            ctshape = ([P, CALLB // 2, 2 * f] if mode == "gather_512"
                       else [P, CALLB, f])
            cts = [[spool.tile(ctshape, mybir.dt.bfloat16,
                               name=f"cb{c}_{i}") for i in range(NBUF)]
                   for c in range(N_CHUNKS)]
            nc.sync.dma_start(out=iota_t[:], in_=iota_d[:])
            nc.sync.dma_start(out=idx_t[:], in_=idx_d[:])
            nc.sync.dma_start(out=val_t[:], in_=val_d[:])
            nc.sync.dma_start(out=rloc_t[:], in_=rloc_d[:])
            nc.gpsimd.load_library(mlp)

            # per-chunk cursor into calls_by_chunk (next unissued call)
            next_call = [0] * N_CHUNKS
            # idxw column offset per call (CALLB batches -> CALLB*8 cols)
            call_col = {}
            acc = 0
            for k, (c_, b0, nb, fw, lw) in enumerate(plan.calls):
                call_col[k] = acc
                acc += nb * 8

            def issue(c, j):
                if mode in ("compute_only", "dve_only", "pe_only"):
                    return
                k, b0, nb, fw, lw = plan.calls_by_chunk[c][j]
                c_t = cts[c][j % NBUF]
                col0 = call_col[k]
                if mode == "gather_512":
                    # timing ablation: same bytes, half the descriptors
                    nbh = nb // 2
                    if nbh:
                        nc.gpsimd.dma_gather(
                            c_t[:, :nbh, :],
                            x_d[:].rearrange("(a b) f -> a (b f)", b=2),
                            idx_t[:, col0:col0 + nbh * 8],
                            nbh * P, nbh * P, 2 * f,
                            single_packet=False,
                        )
                    return
                if mode == "gather_sp":
                    # timing ablation: 1024-idx sub-calls, single_packet=True
                    for s in range(0, nb, 8):
                        sb = min(8, nb - s)
                        nc.gpsimd.dma_gather(
                            c_t[:, s:s + sb, :],
                            x_d[c * plan.chunk_rows:
                                min((c + 1) * plan.chunk_rows, plan.n)],
                            idx_t[:, col0 + s * 8:col0 + (s + sb) * 8],
                            sb * P, sb * P, f,
                        )
                    return
                nc.gpsimd.dma_gather(
                    c_t[:, :nb, :],
                    x_d[c * plan.chunk_rows:
                        min((c + 1) * plan.chunk_rows, plan.n)],
                    idx_t[:, col0:col0 + nb * 8],
                    nb * P, nb * P, f,
                    single_packet=False,
                    queue_num=(c if mode != "gather_q0" else 0),
                )

            def emit_eligible(c, w_done):
                # Emit the next gather call(s) for chunk c.  A call may only
                # be emitted once every consumer of the tile it overwrites
                # (call j-NBUF) has been emitted, i.e. after that call's last
                # reader window has been fully generated (w_done).
                cbc = plan.calls_by_chunk[c]
                while next_call[c] < len(cbc):
                    j = next_call[c]
                    if j >= NBUF and cbc[j - NBUF][4] > w_done:
                        break
                    issue(c, j)
                    next_call[c] += 1

            if mode in ("compute_only", "dve_only", "pe_only"):
                for c in range(N_CHUNKS):
                    for i in range(NBUF):
                        nc.vector.memset(cts[c][i][:], 0.0)
            zst_t = None
            if mode.startswith("gather") or mode == "dve_only":
                zst_t = stpool.tile([P, f], mybir.dt.float32, name="zst")
                nc.vector.memset(zst_t[:], 0.0)
            sconst_t = None
            if mode == "pe_only":
                sconst_t = selpool.tile([P, P], mybir.dt.bfloat16, name="sc")
                nc.vector.memset(sconst_t[:], 0.0)

            for c in range(N_CHUNKS):
                emit_eligible(c, -1)   # prefetch first NBUF calls per chunk

            for w in range(plan.n_windows):
                if mode not in ("compute_only", "dve_only", "pe_only"):
                    for c in range(N_CHUNKS):
                        # the data for window w must already be emitted
                        cbc = plan.calls_by_chunk[c]
                        need = int(plan.gslot[w, c] + plan.bg[w, c] - 1
                                   - plan.chunk_base[c]) // CALLB
                        assert need < next_call[c], (
                            f"w={w} c={c}: call {need} not emitted "
                            f"(next={next_call[c]}); increase NBUF or CALLB")
                if mode.startswith("gather"):
                    nc.sync.dma_start(out=out_d[w * P:(w + 1) * P],
                                      in_=zst_t[:])
                    for c in range(N_CHUNKS):
                        emit_eligible(c, w)
                    continue
                if mode == "dve_only":
                    for c in range(N_CHUNKS):
                        g0 = int(plan.gslot[w, c])
                        for B in range(g0, g0 + int(plan.bg[w, c])):
                            s_t = selpool.tile([P, P], mybir.dt.bfloat16,
                                               name=f"s{B}", tag=f"s{B % 4}")
                            nc.vector.tensor_scalar(
                                out=s_t[:], in0=iota_t[:],
                                scalar1=rloc_t[:, B:B + 1],
                                scalar2=val_t[:, B:B + 1],
                                op0=mybir.AluOpType.is_equal,
                                op1=mybir.AluOpType.mult,
                            )
                    nc.sync.dma_start(out=out_d[w * P:(w + 1) * P],
                                      in_=zst_t[:])
                    continue
                psum_t = ppool.tile([P, f], mybir.dt.float32, name=f"ps{w}",
                                    tag=f"ps{w % 4}", space="PSUM")
                nbat = int(plan.bg[w].sum())
                bi = 0
                for c in range(N_CHUNKS):
                    g0 = int(plan.gslot[w, c])
                    for B in range(g0, g0 + int(plan.bg[w, c])):
                        rel = B - int(plan.chunk_base[c])
                        j, loc = rel // CALLB, rel % CALLB
                        c_t = cts[c][j % NBUF]
                        if mode == "pe_only":
                            s_t = sconst_t
                        else:
                            s_t = selpool.tile([P, P], mybir.dt.bfloat16,
                                               name=f"s{B}", tag=f"s{B % 4}")
                            nc.vector.tensor_scalar(
                                out=s_t[:], in0=iota_t[:],
                                scalar1=rloc_t[:, B:B + 1],
                                scalar2=val_t[:, B:B + 1],
                                op0=mybir.AluOpType.is_equal,
                                op1=mybir.AluOpType.mult,
                            )
                        nc.tensor.matmul(
                            out=psum_t[:], lhsT=s_t[:], rhs=c_t[:, loc, :],
                            start=(bi == 0), stop=(bi == nbat - 1),
                        )
                        bi += 1
                st_t = stpool.tile([P, f], mybir.dt.float32, name=f"st{w}",
                                   tag=f"st{w % 3}")
                nc.scalar.copy(out=st_t[:], in_=psum_t[:])
                nc.sync.dma_start(out=out_d[w * P:(w + 1) * P], in_=st_t[:])
                for c in range(N_CHUNKS):
                    emit_eligible(c, w)
    nc.compile()
    return nc


def _make_inputs(plan, x, per_core):
    iota = np.tile(np.arange(P, dtype=np.float32)[None, :], (P, 1)).astype(BF16)
    xb = np.ascontiguousarray(x.astype(BF16))
    in_maps = []
    for ci in range(N_CORES):
        pc = per_core[ci]
        in_maps.append({
            "x": xb, "iota": iota, "idxw": pc["idxw"], "val": pc["val"],
            "rloc": pc["rloc"],
        })
    return in_maps


def _run(nc, plan, x, per_core):
    from concourse.bass_utils import run_bass_kernel_spmd
    in_maps = _make_inputs(plan, x, per_core)
    res = run_bass_kernel_spmd(nc, in_maps, core_ids=list(range(N_CORES)))
    rpc = plan.rpc
    return np.concatenate(
        [res.results[ci]["out"][:rpc] for ci in range(N_CORES)], axis=0)


_PROGRAM_CACHE = {}


def spmm(x, A_ind, A_val):
    x = np.asarray(x, dtype=np.float32)
    row = np.asarray(A_ind[0], dtype=np.int64)
    col = np.asarray(A_ind[1], dtype=np.int64)
    val = np.asarray(A_val, dtype=np.float32)
    plan, per_core = _plan_and_pack(x, row, col, val)
    key = (x.shape, plan.bg.tobytes())
    nc = _PROGRAM_CACHE.get(key)
    if nc is None:
        nc = _build_program(plan)
        _PROGRAM_CACHE.clear()
        _PROGRAM_CACHE[key] = nc
    return _run(nc, plan, x, per_core)


def kernel(x, A_ind, A_val):
    return spmm(np.asarray(x), np.asarray(A_ind), np.asarray(A_val))
